# revision 23
# baseline (speedup 1.0000x reference)
"""GroupedQueryAttention TRN2 kernel — 8-core SPMD (batch x tensor-parallel).

Sharding: core c = 2*b + tp. Each core handles batch b and kv-heads
{2tp, 2tp+1} (both query groups per kv head co-located). Host folds
mproj into Wk, vproj into Wv, the 1/sqrt(dq) scale into Wq; the k-side
bias is dropped entirely (a per-query constant shift of scores — softmax
invariant), v/o biases become a host-side output constant. Each core
returns a partial y.T [512, T]; host sums the two tp partials per batch
and transposes.

v3 structure (per core, all matmuls bf16, fp32 PSUM):
  - layouts: qq [128, 2T] (rows h*64+d, cols b256*512 + g*256 + m),
    kk [128, T] (rows h*64+d, cols n). Attention runs on 256-wide
    t-blocks with BOTH query groups packed into each matmul's moving
    operand, so every PE instruction has a distinct stationary (no
    redundant LDWEIGHTS) and M=128 is fully used everywhere:
      scores: lhsT = kk[h-rows, n-tile] (base h*64 => PE tile offset),
              rhs = qq 3D view [64, 2, 256-lo], out [128, 2, 256-lo]
      one exp per n-tile; causal mask via gpsimd affine_select
      AV: otg [65, (g,256)-packed] += v_aug x pt (row 64 = denominator)
  - Q/K projections are h-PAIRED: stationary [128,128] covers both kv
    heads -> M=128 (half the matmuls of the per-head version).
  - normalize tail: den row [1,512] spread to [64,8] by SWDGE DMA,
    exact reciprocal there (~16 cols), gather back, ones-matmul
    broadcast deferred into the next PE work window, DVE multiply,
    g=1 shifted to partitions 64:127 via SBUF DMA on the scalar queue.
  - inputs DMA'd as [128,1024] half-chunks: qt/kt on the sync queue,
    vt/weights on the scalar queue; ACT engine runs Exp only.
"""

import numpy as np
import ml_dtypes

import concourse.bass as bass
import concourse.bacc as bacc
import concourse.mybir as mybir
from concourse import tile
from concourse.bass_utils import run_bass_kernel_spmd

B, T, D = 4, 2048, 512
HQ, HKV = 8, 4
DQ, DKV = 64, 128
G = 2
NCORES = 8
BF16NP = ml_dtypes.bfloat16

f32 = mybir.dt.float32
bf16 = mybir.dt.bfloat16
EXP = mybir.ActivationFunctionType.Exp
GE = mybir.AluOpType.is_ge


def build_module(t=T, debug_outs=False):
    assert t % 1024 == 0
    tb_n = t // 512    # 512-wide t blocks (phase1/phase3 granularity)
    tq_n = t // 256    # 256-wide t blocks (phase2 granularity)
    nt_n = t // 128    # 128-wide n tiles
    hf = t // 2

    nc = bacc.Bacc("TRN2", target_bir_lowering=False, debug=False)

    qt_d = nc.dram_tensor("qt", [512, t], bf16, kind="ExternalInput").ap()
    kt_d = nc.dram_tensor("kt", [512, t], bf16, kind="ExternalInput").ap()
    vt_d = nc.dram_tensor("vt", [512, t], bf16, kind="ExternalInput").ap()
    wq_d = nc.dram_tensor("wq", [512, 256], bf16, kind="ExternalInput").ap()
    wk_d = nc.dram_tensor("wk", [512, 128], bf16, kind="ExternalInput").ap()
    wv_d = nc.dram_tensor("wv", [512, 130], bf16, kind="ExternalInput").ap()
    wo_d = nc.dram_tensor("wo", [256, 512], bf16, kind="ExternalInput").ap()
    bq_d = nc.dram_tensor("bq", [128, 2], f32, kind="ExternalInput").ap()
    yt_d = nc.dram_tensor("yt", [512, t], bf16, kind="ExternalOutput").ap()
    if debug_outs:
        dbg = {k: nc.dram_tensor(k, sh, bf16, kind="ExternalOutput").ap()
               for k, sh in [("dqq", [128, 2 * t]), ("dkk", [128, t]),
                             ("dv", [128, (t // 128) * 130]),
                             ("do0", [128, t]), ("do1", [128, t])]}

    with tile.TileContext(nc) as tc:
        with tc.tile_pool(name="const", bufs=1) as cpool, \
             tc.tile_pool(name="big", bufs=1) as bigp:
            wq_sb = cpool.tile([128, 4 * 256], bf16, tag="wq", name="wq")
            wk_sb = cpool.tile([128, 4 * 128], bf16, tag="wk", name="wk")
            wv_sb = cpool.tile([128, 4 * 130], bf16, tag="wv", name="wv")
            wo_sb = cpool.tile([128, 2 * 512], bf16, tag="wo", name="wo")
            bq_sb = cpool.tile([128, 2], f32, tag="bq", name="bq")
            ones_sb = cpool.tile([65, 128], bf16, tag="ones", name="ones")

            qt_sb = bigp.tile([128, 4 * t], bf16, tag="qt", name="qt")
            kt_sb = bigp.tile([128, 4 * t], bf16, tag="kt", name="kt")
            vt_sb = bigp.tile([128, 4 * t], bf16, tag="vt", name="vt")

            # --- input DMA: one 3D-AP descriptor per (tensor, half).
            # qt/kt on sync; vt + weights on gpsimd (SWDGE) so the
            # scalar engine runs Exp only.
            def chunked(ap2d, c):
                return ap2d.rearrange("(c p) m -> p c m", c=c)

            def half_view(sb, half):
                return sb[:, :].rearrange("p (c m) -> p c m", c=4)[
                    :, :, half * hf:(half + 1) * hf]

            nc.sync.dma_start(bq_sb[:, :], bq_d[:, :])
            nc.sync.dma_start(
                wq_sb[:, :].rearrange("p (c m) -> p c m", c=4),
                chunked(wq_d[:, :], 4))
            nc.gpsimd.dma_start(
                wv_sb[:, :].rearrange("p (c m) -> p c m", c=4),
                chunked(wv_d[:, :], 4))
            nc.sync.dma_start(half_view(qt_sb, 0), chunked(qt_d[:, 0:hf], 4))
            nc.sync.dma_start(
                wk_sb[:, :].rearrange("p (c m) -> p c m", c=4),
                chunked(wk_d[:, :], 4))
            nc.gpsimd.dma_start(half_view(vt_sb, 0), chunked(vt_d[:, 0:hf], 4))
            nc.sync.dma_start(half_view(kt_sb, 0), chunked(kt_d[:, 0:hf], 4))
            nc.gpsimd.dma_start(
                wo_sb[:, :].rearrange("p (c m) -> p c m", c=2),
                chunked(wo_d[:, :], 2))
            nc.sync.dma_start(half_view(qt_sb, 1), chunked(qt_d[:, hf:t], 4))
            nc.sync.dma_start(half_view(kt_sb, 1), chunked(kt_d[:, hf:t], 4))
            nc.gpsimd.dma_start(half_view(vt_sb, 1), chunked(vt_d[:, hf:t], 4))
            nc.vector.memset(ones_sb[64:65, :], 1.0)

            qq = bigp.tile([128, 2 * t], bf16, tag="qq", name="qq")
            kk = bigp.tile([128, t], bf16, tag="kk", name="kk")
            v_sb = bigp.tile([128, nt_n * 130], bf16, tag="v", name="v")
            oT = [bigp.tile([128, t], bf16, tag=f"oT{h}", name=f"oT{h}")
                  for h in range(2)]

            with tc.tile_pool(name="p1", bufs=2, space="PSUM") as p1, \
                 tc.tile_pool(name="s2", bufs=3, space="PSUM") as s2p, \
                 tc.tile_pool(name="otp", bufs=1, space="PSUM") as otp, \
                 tc.tile_pool(name="bcp", bufs=1, space="PSUM") as bcp, \
                 tc.tile_pool(name="ptp", bufs=6) as ptp, \
                 tc.tile_pool(name="npool", bufs=2) as npl, \
                 tc.tile_pool(name="ysp", bufs=4) as ysp:

                pending = []  # deferred tail closures (PE bc matmuls + muls)
                workq = []    # deferred phase1 group closures (PE filler)

                def flush_tail(n=None):
                    k = len(pending) if n is None else min(n, len(pending))
                    for _ in range(k):
                        pending.pop(0)()

                def pump(k=1):
                    for _ in range(min(k, len(workq))):
                        workq.pop(0)()

                def drain_workq():
                    while workq:
                        workq.pop(0)()

                def q_group(tb, g):
                    def em():
                        ps = p1.tile([128, 512], f32, tag="p1", name="p1")
                        for c in range(4):
                            nc.tensor.matmul(
                                ps[:, :],
                                wq_sb[:, c * 256 + g * 128:
                                      c * 256 + (g + 1) * 128],
                                qt_sb[:, c * t + tb * 512: c * t + (tb + 1) * 512],
                                start=(c == 0), stop=(c == 3))
                        qv = qq[:, tb * 1024:(tb + 1) * 1024].rearrange(
                            "p (b r) -> p b r", b=2)[:, :, g * 256:(g + 1) * 256]
                        nc.vector.tensor_scalar_add(
                            qv, ps[:, :].rearrange("p (b m) -> p b m", b=2),
                            bq_sb[:, g:g + 1])
                    return em

                def k_group(tb):
                    def em():
                        ps = p1.tile([128, 512], f32, tag="p1", name="p1")
                        for c in range(4):
                            nc.tensor.matmul(
                                ps[:, :],
                                wk_sb[:, c * 128:(c + 1) * 128],
                                kt_sb[:, c * t + tb * 512: c * t + (tb + 1) * 512],
                                start=(c == 0), stop=(c == 3))
                        nc.vector.tensor_copy(
                            kk[:, tb * 512:(tb + 1) * 512], ps[:, :])
                    return em

                def v_group(nt):
                    def em():
                        ps = p1.tile([128, 512], f32, tag="p1", name="p1")
                        for c in range(4):
                            nc.tensor.matmul(
                                ps[:, 0:130],
                                vt_sb[:, c * t + nt * 128: c * t + (nt + 1) * 128],
                                wv_sb[:, c * 130:(c + 1) * 130],
                                start=(c == 0), stop=(c == 3))
                        nc.vector.tensor_copy(v_sb[:, nt * 130:(nt + 1) * 130],
                                              ps[:, 0:130])
                        nc.gpsimd.memset(
                            v_sb[:, nt * 130 + 64: nt * 130 + 65], 1.0)
                        nc.gpsimd.memset(
                            v_sb[:, nt * 130 + 129: nt * 130 + 130], 1.0)
                    return em

                def phase1_groups(tb):
                    return ([q_group(tb, g) for g in range(2)]
                            + [k_group(tb)]
                            + [v_group(4 * tb + j) for j in range(4)])

                def phase1(tb):
                    for i, em in enumerate(phase1_groups(tb)):
                        em()
                        if i == 1:
                            flush_tail()

                def emit_scores(h, tq, i, pt_tiles):
                    """scores matmul + exp + mask for n-tile i (g-packed)."""
                    t0 = tq * 256
                    n0 = 128 * i
                    lo = max(0, n0 - t0)   # 0 or 128
                    s2 = s2p.tile([128, 512], f32, tag="s2", name="s2")
                    s2v = s2[:, :].rearrange("p (g m) -> p g m", g=2)[:, :, lo:256]
                    qv = qq[h * 64:(h + 1) * 64,
                            tq * 512:(tq + 1) * 512].rearrange(
                        "p (g m) -> p g m", g=2)[:, :, lo:256]
                    nc.tensor.matmul(s2v, kk[h * 64:(h + 1) * 64, n0:n0 + 128],
                                     qv, start=True, stop=True)
                    pt = ptp.tile([128, 512], bf16, tag="pt", name="pt")
                    ptv = pt[:, :].rearrange("p (g m) -> p g m", g=2)[:, :, lo:256]
                    nc.scalar.activation(ptv, s2v, EXP)
                    if n0 >= t0:
                        for g in range(2):
                            sl = pt[:, g * 256 + lo: g * 256 + lo + 128]
                            nc.gpsimd.affine_select(
                                out=sl, in_=sl, compare_op=GE, fill=0.0,
                                base=0, pattern=[[1, 128]],
                                channel_multiplier=-1)
                    pt_tiles[(h, i)] = pt

                def emit_av(h, tq, i, nch, otg, pt_tiles):
                    t0 = tq * 256
                    n0 = 128 * i
                    lo = max(0, n0 - t0)
                    pt = pt_tiles[(h, i)]
                    ptv = pt[:, :].rearrange("p (g m) -> p g m", g=2)[:, :, lo:256]
                    ov = otg[:, :].rearrange("p (g m) -> p g m", g=2)[:, :, lo:256]
                    nc.tensor.matmul(
                        ov, v_sb[:, i * 130 + h * 65: i * 130 + h * 65 + 65],
                        ptv, start=(i == 0), stop=(i == nch - 1),
                        skip_group_check=True)
                    pt_tiles[(h, i)] = None

                def phase2(tq):
                    # both kv heads interleaved: scores for h=0 (PE rows
                    # 0:64) and h=1 (rows 64:128) are disjoint row groups,
                    # so their LDWEIGHTS overlap the running matmul.
                    t0 = tq * 256
                    tsl = slice(t0, t0 + 256)
                    nch = 2 * (tq + 1)
                    otg = {h: otp.tile([65, 512], f32, tag=f"ot{h}",
                                       name=f"ot{h}") for h in range(2)}
                    pt_tiles = {}
                    for h in range(2):
                        emit_scores(h, tq, 0, pt_tiles)
                    for i in range(nch):
                        if i + 1 < nch:
                            for h in range(2):
                                emit_scores(h, tq, i + 1, pt_tiles)
                        if i == 2 or i == 3:
                            flush_tail(1)
                        elif i > 0:
                            pump(1)
                        for h in range(2):
                            emit_av(h, tq, i, nch, otg[h], pt_tiles)
                    # ---- tails: normalize both h ----
                    dsb = npl.tile([65, 1024], f32, tag="dsb", name="dsb")
                    dsp = npl.tile([64, 32], f32, tag="dsp", name="dsp")
                    rcb = npl.tile([64, 16], bf16, tag="rcb", name="rcb")
                    rb = npl.tile([65, 1024], bf16, tag="rb", name="rb")
                    nm = {h: npl.tile([64, 512], bf16, tag=f"nm{h}",
                                      name=f"nm{h}") for h in range(2)}
                    for h in range(2):
                        nc.vector.tensor_copy(dsb[64:65, h * 512:(h + 1) * 512],
                                              otg[h][64:65, :])
                        nc.vector.tensor_copy(nm[h][:, :], otg[h][0:64, :])
                        nc.gpsimd.dma_start(dsp[:, h * 8:(h + 1) * 8],
                                            dsb[64:65, h * 512:(h + 1) * 512])
                    nc.vector.reciprocal(dsp[:, 16:32], dsp[:, 0:16])
                    nc.vector.tensor_copy(rcb[:, :], dsp[:, 16:32])
                    for h in range(2):
                        nc.gpsimd.dma_start(rb[64:65, h * 512:(h + 1) * 512],
                                            rcb[:, h * 8:(h + 1) * 8])

                    def mk_tail(h):
                        def tail(h=h, tsl=tsl, rb=rb, nm=nm):
                            bc = bcp.tile([64, 512], f32, tag="bc", name="bc")
                            nc.tensor.matmul(
                                bc[:, :], ones_sb[64:65, 0:64],
                                rb[64:65, h * 512:(h + 1) * 512],
                                start=True, stop=True)
                            nc.vector.tensor_mul(oT[h][0:64, tsl],
                                                 nm[h][:, 0:256], bc[:, 0:256])
                            nm1 = npl.tile([64, 256], bf16, tag=f"nm1b{h}",
                                           name=f"nm1b{h}")
                            nc.vector.tensor_mul(nm1[:, :], nm[h][:, 256:512],
                                                 bc[:, 256:512])
                            nc.gpsimd.dma_start(oT[h][64:128, tsl], nm1[:, :])
                        return tail

                    pending.append(mk_tail(0))
                    pending.append(mk_tail(1))
                    # tails flushed by next phase2 / phase1 / phase3

                def phase3(tb):
                    tsl = slice(tb * 512, (tb + 1) * 512)
                    flush_tail()
                    for oc in range(4):
                        yp = p1.tile([128, 512], f32, tag="p1", name="p1")
                        for hh in range(2):
                            nc.tensor.matmul(
                                yp[:, :],
                                wo_sb[:, hh * 512 + oc * 128:
                                      hh * 512 + (oc + 1) * 128],
                                oT[hh][:, tsl],
                                start=(hh == 0), stop=(hh == 1))
                        ys = ysp.tile([128, 512], bf16, tag="ys", name="ys")
                        nc.vector.tensor_copy(ys[:, :], yp[:, :])
                        nc.sync.dma_start(
                            yt_d[oc * 128:(oc + 1) * 128, tsl], ys[:, :])

                phase1(0)
                for tbb in range(tb_n):
                    if tbb + 1 < tb_n:
                        workq.extend(phase1_groups(tbb + 1))
                    phase2(2 * tbb)
                    phase2(2 * tbb + 1)
                    drain_workq()
                    phase3(tbb)
                flush_tail()
                if debug_outs:
                    nc.sync.dma_start(dbg["dqq"][:, :], qq[:, :])
                    nc.sync.dma_start(dbg["dkk"][:, :], kk[:, :])
                    for h in range(2):
                        nc.sync.dma_start(dbg[f"do{h}"][:, :], oT[h][:, :])
                    nc.sync.dma_start(dbg["dv"][:, :], v_sb[:, :])

    nc.compile()
    return nc


def prep_inputs(inputs, t=T):
    """Host-side fold + shard. Returns (in_maps[8], out_const[512] f32)."""
    Q = np.asarray(inputs["Q"], np.float32)
    K = np.asarray(inputs["K"], np.float32)
    V = np.asarray(inputs["V"], np.float32)
    Wq_w = np.asarray(inputs["Wq_w"], np.float32)
    Wq_b = np.asarray(inputs["Wq_b"], np.float32)
    Wk_w = np.asarray(inputs["Wk_w"], np.float32)
    Wv_w = np.asarray(inputs["Wv_w"], np.float32)
    Wv_b = np.asarray(inputs["Wv_b"], np.float32)
    Wo_w = np.asarray(inputs["Wo_w"], np.float32)
    Wo_b = np.asarray(inputs["Wo_b"], np.float32)
    vproj_w = np.asarray(inputs["vproj_w"], np.float32)
    vproj_b = np.asarray(inputs["vproj_b"], np.float32)
    mproj_w = np.asarray(inputs["mproj_w"], np.float32)

    b_n = Q.shape[0]
    s = 1.0 / np.sqrt(np.float32(DQ))

    qt = [np.ascontiguousarray(Q[b, :t].T).astype(BF16NP) for b in range(b_n)]
    kt = [np.ascontiguousarray(K[b, :t].T).astype(BF16NP) for b in range(b_n)]
    vt = [np.ascontiguousarray(V[b, :t].T).astype(BF16NP) for b in range(b_n)]

    per_tp = []
    for tp in range(2):
        wq = np.zeros((512, 256), np.float32)   # cols: g*128 + h*64 + d
        bq = np.zeros((128, 2), np.float32)     # [h*64+d, g]
        wk = np.zeros((512, 128), np.float32)   # cols: h*64 + d
        wv = np.zeros((512, 130), np.float32)
        wo = np.zeros((256, 512), np.float32)   # rows: (2h+g)*64 + d
        for h in range(2):
            hg = 2 * tp + h
            wk_eff = mproj_w @ Wk_w[hg * 128:(hg + 1) * 128, :]
            wk[:, h * 64:(h + 1) * 64] = wk_eff.T
            wv_eff = vproj_w @ Wv_w[hg * 128:(hg + 1) * 128, :]
            wv[:, h * 65:h * 65 + 64] = wv_eff.T
            for g in range(2):
                p = 2 * h + g
                hq = g * HKV + hg
                wq[:, g * 128 + h * 64: g * 128 + (h + 1) * 64] = \
                    (Wq_w[hq * 64:(hq + 1) * 64, :] * s).T
                bq[h * 64:(h + 1) * 64, g] = Wq_b[hq * 64:(hq + 1) * 64] * s
                wo[p * 64:(p + 1) * 64, :] = Wo_w[:, hq * 64:(hq + 1) * 64].T
        per_tp.append(dict(
            wq=wq.astype(BF16NP), wk=wk.astype(BF16NP), wv=wv.astype(BF16NP),
            wo=wo.astype(BF16NP), bq=bq))

    out_const = Wo_b.copy()
    for hq in range(HQ):
        hg = hq % HKV
        bv_eff = vproj_w @ Wv_b[hg * 128:(hg + 1) * 128] + vproj_b
        out_const += Wo_w[:, hq * 64:(hq + 1) * 64] @ bv_eff

    in_maps = []
    for b in range(b_n):
        for tp in range(2):
            w = per_tp[tp]
            in_maps.append(dict(
                qt=qt[b], kt=kt[b], vt=vt[b],
                wq=w["wq"], wk=w["wk"], wv=w["wv"], wo=w["wo"],
                bq=w["bq"]))
    return in_maps, out_const


_NC_CACHE = {}


def get_module(t=T):
    key = t
    if key not in _NC_CACHE:
        _NC_CACHE[key] = build_module(t)
    return _NC_CACHE[key]


def run_on_cores(inputs, t=T, **run_kwargs):
    nc = get_module(t)
    in_maps, out_const = prep_inputs(inputs, t)
    res = run_bass_kernel_spmd(nc, in_maps, core_ids=list(range(NCORES)),
                               **run_kwargs)
    b_n = len(in_maps) // 2
    out = np.empty((b_n, t, D), np.float32)
    for b in range(b_n):
        acc = (res.results[2 * b]["yt"].astype(np.float32)
               + res.results[2 * b + 1]["yt"].astype(np.float32))
        out[b] = acc.T + out_const[None, :]
    return out, res


def kernel(**inputs):
    out, _ = run_on_cores(inputs, t=T)
    return out
